# revision 17
# baseline (speedup 1.0000x reference)
import sys

if "/opt/trn_rl_repo" not in sys.path:
    sys.path.insert(0, "/opt/trn_rl_repo")

import os
import numpy as np
import ml_dtypes

BF16 = ml_dtypes.bfloat16
NO_CC = os.environ.get("KNO_CC", "0") == "1"   # debug: skip collectives (sim)

# Problem constants (nn_BiLSTM_77034533421798)
T_FULL = 512
B_FULL = 128
H = 400
G = 1600
BL = 32            # batch lanes per core: core = (quarter q, direction d)
HP = 512           # padded hidden dim: 4 chunks x 128 (units 400:512 are zero)
NEG = 60000.0  # fits fp16; sigmoid(-60000)=0

# PSUM/stream gate-column layout: [g | i | f | o], 400 each. PSUM blocks are
# placed at 512-col offsets so each gate block sits in its own 2KB bank.
PT2 = [2, 0, 1, 3]  # block (g,i,f,o) -> pytorch gate-block (i,f,g,o) index
PERM = np.empty(G, np.int64)
for _n in range(G):
    PERM[_n] = PT2[_n // 400] * 400 + _n % 400
SIGMA = np.arange(G) >= 400  # i,f,o get the NEG mask row; g does not

USTEP = 8   # steps per For_i body (2 groups of 4)
GRP = 4     # steps per xg-GEMM group (128 rows)


def _build_program(T):
    import concourse.bacc as bacc
    import concourse.mybir as mybir
    import concourse.bass as bass
    import concourse.tile as tile

    ds = bass.ds
    dt = mybir.dt
    TB = T * BL
    EB = min(64, T)
    NBLK = T // EB
    assert T % EB == 0 and EB % USTEP == 0

    nc = bacc.Bacc("TRN2", target_bir_lowering=False, debug=False, num_devices=8)

    # ---------------- I/O (all bf16 to minimize axon transfer) ----------------
    XTAP = nc.dram_tensor("XTAP", [128, 4, TB + 128], dt.float16, kind="ExternalInput")
    W0SP = nc.dram_tensor("W0SP", [4, 128, G], dt.float16, kind="ExternalInput")
    WH0P = nc.dram_tensor("WH0P", [4, 128, G], dt.float16, kind="ExternalInput")
    W1SP = nc.dram_tensor("W1SP", [8, 128, G], dt.float16, kind="ExternalInput")
    W1M = nc.dram_tensor("W1M", [2, G], dt.float16, kind="ExternalInput")
    WH1P = nc.dram_tensor("WH1P", [4, 128, G], dt.float16, kind="ExternalInput")
    MOI = nc.dram_tensor("MOI", [2, TB + 128], dt.float16, kind="ExternalInput")
    I32 = nc.dram_tensor("I32", [128, 32], dt.float16, kind="ExternalInput")
    SEL0 = nc.dram_tensor("SEL0", [128, 1], dt.float32, kind="ExternalInput")
    SEL1 = nc.dram_tensor("SEL1", [128, 1], dt.float32, kind="ExternalInput")
    OUT = nc.dram_tensor("OUT", [TB, H], dt.float16, kind="ExternalOutput")

    Sigmoid = mybir.ActivationFunctionType.Sigmoid
    Tanh = mybir.ActivationFunctionType.Tanh

    with tile.TileContext(nc) as tc:
        with (
            tc.tile_pool(name="dram", bufs=1, space="DRAM") as dp,
            tc.tile_pool(name="wres", bufs=1) as wres,
            tc.tile_pool(name="cst", bufs=1) as cst,
            tc.tile_pool(name="state", bufs=1) as stp,
            tc.tile_pool(name="stat", bufs=2) as statp,
            tc.tile_pool(name="xgp", bufs=1) as xgp,
            tc.tile_pool(name="wk", bufs=2) as wk,
            tc.tile_pool(name="pacc", bufs=1, space="PSUM") as pacc,
            tc.tile_pool(name="pgem", bufs=2, space="PSUM") as pgem,
            tc.tile_pool(name="ptr", bufs=2, space="PSUM") as ptr,
        ):
            # internal DRAM
            L0T = dp.tile([128, 4, TB + 128], dt.float16, tag="L0T", name="L0T")
            EXIN = dp.tile([NBLK, 128, 4, EB * BL], dt.float16, tag="EXIN", name="EXIN")
            EXOB = dp.tile([NBLK, 2, 128, 4, EB * BL], dt.float16, tag="EXOB", name="EXOB")
            EXF = dp.tile([2, 128, 4, TB + 128], dt.float16, tag="EXF", name="EXF")

            # ---- residents ----
            i32 = cst.tile([128, 32], dt.float16, tag="i32")
            nc.sync.dma_start(out=i32[:], in_=I32[:])
            sel0 = cst.tile([128, 1], dt.float32, tag="sel0")
            nc.sync.dma_start(out=sel0[:], in_=SEL0[:])
            sel1 = cst.tile([128, 1], dt.float32, tag="sel1")
            nc.sync.dma_start(out=sel1[:], in_=SEL1[:])
            w1m = cst.tile([2, G], dt.float16, tag="w1m")
            nc.sync.dma_start(out=w1m[:], in_=W1M[:])

            def load_wchunks(src, n, tagp, wdt):
                out = []
                for k in range(n):
                    w = wres.tile([128, G], wdt, tag=f"{tagp}{k}")
                    nc.sync.dma_start(out=w[:], in_=src[k])
                    out.append(w)
                return out

            w0c = load_wchunks(W0SP, 4, "w0c", dt.float16)
            wh0c = load_wchunks(WH0P, 4, "wh0c", dt.float16)
            w1c = load_wchunks(W1SP, 8, "w1c", dt.float16)
            wh1c = load_wchunks(WH1P, 4, "wh1c", dt.float16)

            # xg double buffers (SBUF-resident, no DRAM round trip)
            xgA = xgp.tile([128, G], dt.float16, tag="xgA")
            xgB = xgp.tile([128, G], dt.float16, tag="xgB")

            # persistent recurrence state (ping-pong A/B)
            cS = [stp.tile([BL, H], dt.float32, tag=f"c{i}", name=f"c{i}")
                  for i in range(2)]
            hTS = [stp.tile([128, 128], dt.float16, tag=f"hT{i}", name=f"hT{i}")
                   for i in range(2)]
            hbS = [stp.tile([BL, HP], dt.float16, tag=f"hb{i}", name=f"hb{i}")
                   for i in range(2)]

            # ---------------- helpers ----------------
            def gemm0(row0, xgt):
                """layer-0 xg GEMM for 4 steps (128 rows at dynamic row0)."""
                st = statp.tile([128, 4, 128], dt.float16, tag="st0")
                nc.sync.dma_start(out=st[:], in_=XTAP[:, :, ds(row0, 128)])
                for b in range(4):
                    acc = pgem.tile([128, 400], dt.float32, tag="gacc")
                    for k in range(4):
                        nc.tensor.matmul(
                            acc[:], st[:, k, :], w0c[k][:, 400 * b:400 * (b + 1)],
                            start=(k == 0), stop=(k == 3))
                    nc.vector.tensor_copy(xgt[:, 400 * b:400 * (b + 1)], acc[:])

            def gemm1(row0, xgt):
                """layer-1 xg GEMM for 4 steps: own(4) + peer(4) + mask/bias."""
                own = statp.tile([128, 4, 128], dt.float16, tag="own1")
                nc.sync.dma_start(out=own[:], in_=L0T[:, :, ds(row0, 128)])
                e0 = statp.tile([128, 4, 128], dt.float16, tag="e0")
                nc.sync.dma_start(out=e0[:], in_=EXF[0, :, :, ds(row0, 128)])
                e1 = statp.tile([128, 4, 128], dt.float16, tag="e1")
                nc.sync.dma_start(out=e1[:], in_=EXF[1, :, :, ds(row0, 128)])
                pm = statp.tile([128, 4, 128], dt.float16, tag="pm")
                nc.vector.tensor_scalar_mul(pm[:], e1[:], sel1[:])
                nc.vector.scalar_tensor_tensor(
                    pm[:], e0[:], sel0[:], pm[:],
                    mybir.AluOpType.mult, mybir.AluOpType.add)
                nc.sync.dma_start(out=pm[16:18, 3, :], in_=MOI[:, ds(row0, 128)])
                for b in range(4):
                    acc = pgem.tile([128, 400], dt.float32, tag="gacc")
                    for k in range(4):
                        nc.tensor.matmul(
                            acc[:], own[:, k, :], w1c[k][:, 400 * b:400 * (b + 1)],
                            start=(k == 0), stop=False)
                    for k in range(4):
                        nc.tensor.matmul(
                            acc[:], pm[:, k, :], w1c[4 + k][:, 400 * b:400 * (b + 1)],
                            start=False, stop=(k == 3))
                    nc.vector.tensor_copy(xgt[:, 400 * b:400 * (b + 1)], acc[:])

            def step(par, xgt, srow, whc, iv, s, blk, write_l0, write_out):
                """One recurrence step. par: parity (in=par, out=1-par).
                iv: loop var (step index base), s: python offset in body."""
                c_in, c_out = cS[par], cS[1 - par]
                hT_in, hT_out = hTS[par], hTS[1 - par]
                hb_out = hbS[1 - par]

                accs = [pacc.tile([BL, 400], dt.float32, tag=f"acc{b}",
                                  name=f"acc{b}") for b in range(4)]
                xr = xgt[srow * BL:(srow + 1) * BL, :]
                # deposit xg into psum banks (identity matmul), then hh;
                # one psum tile per gate bank so each gate's ACT can start as
                # soon as its own accumulation group completes
                idt = i32[srow * BL:(srow + 1) * BL, :]
                BORD = (2, 1, 0, 3)   # completion order: f, i, g, o
                for b in BORD:
                    nc.tensor.matmul(accs[b][:], idt,
                                     xr[:, 400 * b:400 * (b + 1)],
                                     start=True, stop=False,
                                     tile_position=(srow * BL, 0))
                for b in BORD:
                    for k in range(4):
                        nc.tensor.matmul(
                            accs[b][:],
                            hT_in[:, 32 * k:32 * (k + 1)],
                            whc[k][:, 400 * b:400 * (b + 1)],
                            start=False, stop=(k == 3))

                gt = wk.tile([BL, 400], dt.float32, tag="gt")
                sig = wk.tile([BL, 1200], dt.float32, tag="sig")
                tct = wk.tile([BL, 400], dt.float32, tag="tct")
                t1 = wk.tile([BL, 400], dt.float32, tag="t1")
                t2 = wk.tile([BL, 400], dt.float32, tag="t2")

                nc.scalar.activation(sig[:, 400:800], accs[2][:], Sigmoid)
                nc.scalar.activation(sig[:, 0:400], accs[1][:], Sigmoid)
                nc.scalar.activation(gt[:], accs[0][:], Tanh)
                nc.scalar.activation(sig[:, 800:1200], accs[3][:], Sigmoid)
                nc.gpsimd.tensor_mul(t1[:], sig[:, 400:800], c_in[:])
                nc.vector.tensor_mul(t2[:], sig[:, 0:400], gt[:])
                nc.vector.tensor_add(c_out[:], t1[:], t2[:])
                nc.scalar.activation(tct[:], c_out[:], Tanh)
                nc.vector.tensor_mul(hb_out[:, 0:400], sig[:, 800:1200], tct[:])

                # transpose h -> hT via PE (4 x [32,128] -> [128,32])
                tp = ptr.tile([128, 128], dt.float16, tag="tp")
                for cch in range(4):
                    nc.tensor.transpose(tp[:, 32 * cch:32 * (cch + 1)],
                                        hb_out[:, 128 * cch:128 * (cch + 1)],
                                        i32[0:32, :])
                nc.vector.tensor_copy(hT_out[:], tp[:])

                if write_l0:
                    col = iv * BL + s * BL
                    nc.sync.dma_start(out=L0T[:, :, ds(col, BL)], in_=hT_out[:])
                    # reversed store: rt = T-1-t; jb = NBLK-1-blk (static);
                    # jo*BL = (EB-1 - (iv - blk*EB + s)) * BL
                    rcol = (EB - 1 - s + blk * EB) * BL - iv * BL
                    nc.sync.dma_start(out=EXIN[NBLK - 1 - blk][:, :, ds(rcol, BL)],
                                      in_=hT_out[:])
                if write_out:
                    col = iv * BL + s * BL
                    nc.sync.dma_start(out=OUT[ds(col, BL), :], in_=hb_out[:, 0:H])

            def recurrence(whc, gemm, write_l0, write_out):
                nc.vector.memset(cS[0][:], 0.0)
                nc.vector.memset(hTS[0][:], 0.0)
                nc.vector.memset(hbS[0][:], 0.0)
                nc.vector.memset(hbS[1][:], 0.0)
                gemm(0, xgA)  # prologue: first group
                for blk in range(NBLK):
                    with tc.For_i(blk * EB, (blk + 1) * EB, USTEP) as iv:
                        for s in range(USTEP // 2):
                            step(s % 2, xgA, s, whc, iv, s, blk, write_l0, write_out)
                        gemm(iv * BL + 4 * BL, xgB)
                        for s in range(USTEP // 2, USTEP):
                            step(s % 2, xgB, s - 4, whc, iv, s, blk, write_l0, write_out)
                        gemm(iv * BL + 8 * BL, xgA)
                    if write_l0:
                        jb = NBLK - 1 - blk
                        if not NO_CC:
                            nc.gpsimd.collective_compute(
                                "AllGather", mybir.AluOpType.bypass,
                                replica_groups=[[0, 1], [2, 3], [4, 5], [6, 7]],
                                ins=[EXIN[jb]], outs=[EXOB[jb]],
                            )
                        nc.sync.dma_start(
                            out=EXF[:, :, :, jb * EB * BL:(jb + 1) * EB * BL],
                            in_=EXOB[jb])

            recurrence(wh0c, gemm0, write_l0=True, write_out=False)
            recurrence(wh1c, gemm1, write_l0=False, write_out=True)

    nc.compile()
    return nc


_PROG_CACHE = {}


def _get_program(T):
    if T not in _PROG_CACHE:
        _PROG_CACHE[T] = _build_program(T)
    return _PROG_CACHE[T]


def _prep_core_inputs(x, lengths, wdict, T):
    """Per-core input maps. x: [T,B,400] f32, lengths: [B] int."""
    B = x.shape[1]
    TB = T * BL
    mask = (np.arange(T)[:, None] < np.asarray(lengths)[None, :]).astype(np.float32)
    i32 = np.eye(32, dtype=np.float32)
    in_maps = []
    for core in range(8):
        q, d = core // 2, core % 2
        bs = slice(BL * q, BL * (q + 1))
        xl = np.asarray(x[:, bs, :], np.float32)
        ml = mask[:, bs]
        if d:
            xl, ml = xl[::-1], ml[::-1]
        xt = xl.reshape(TB, 400).T                        # [400, TB]
        negrow = NEG * (1.0 - ml).reshape(1, TB)
        XTAP = np.zeros((512, TB + 128), np.float32)
        XTAP[:400, :TB] = xt
        XTAP[400, :TB] = negrow
        XTAP[401, :TB] = 1.0
        MOI = np.zeros((2, TB + 128), np.float32)
        MOI[0, :TB] = negrow
        MOI[1, :TB] = 1.0

        dd = "f" if d == 0 else "b"
        wi0, whh0 = wdict[f"w_ih_{dd}0"], wdict[f"w_hh_{dd}0"]
        bi0 = wdict[f"b_ih_{dd}0"] + wdict[f"b_hh_{dd}0"]
        wi1, whh1 = wdict[f"w_ih_{dd}1"], wdict[f"w_hh_{dd}1"]
        bi1 = wdict[f"b_ih_{dd}1"] + wdict[f"b_hh_{dd}1"]

        W0SP = np.zeros((512, G), np.float32)
        W0SP[:400] = wi0[PERM, :].T
        W0SP[400] = np.where(SIGMA, -1.0, 0.0)
        W0SP[401] = bi0[PERM]

        WH0P = np.zeros((512, G), np.float32)
        WH0P[:400] = whh0[PERM, :].T

        W1SP = np.zeros((1024, G), np.float32)
        W1SP[0:400] = wi1[:, 400 * d:400 * d + 400][PERM, :].T
        W1SP[512:912] = wi1[:, 400 * (1 - d):400 * (1 - d) + 400][PERM, :].T
        W1M = np.zeros((2, G), np.float32)
        W1M[0] = np.where(SIGMA, -1.0, 0.0)
        W1M[1] = bi1[PERM]
        # fold mask/bias stream rows into peer chunk3's zero pad (rows 912:914
        # of the flat [1024, G] = chunk 7 rows 16:18)
        W1SP[912:914] = W1M

        WH1P = np.zeros((512, G), np.float32)
        WH1P[:400] = whh1[PERM, :].T

        in_maps.append({
            "XTAP": np.ascontiguousarray(
                XTAP.reshape(4, 128, TB + 128).transpose(1, 0, 2)).astype(np.float16),
            "W0SP": W0SP.reshape(4, 128, G).astype(np.float16),
            "WH0P": WH0P.reshape(4, 128, G).astype(np.float16),
            "W1SP": W1SP.reshape(8, 128, G).astype(np.float16),
            "W1M": W1M.astype(np.float16),
            "WH1P": WH1P.reshape(4, 128, G).astype(np.float16),
            "MOI": MOI.astype(np.float16),
            "I32": np.tile(i32, (4, 1)).astype(np.float16),
            "SEL0": np.full((128, 1), float(d), np.float32),
            "SEL1": np.full((128, 1), 1.0 - float(d), np.float32),
        })
    return in_maps


# ---------------- cached PJRT runner ----------------
_CALL_CACHE = {}


def _spmd_call(nc, in_maps):
    """Like run_bass_kernel_spmd under axon, but caches the jitted executable
    per program so steady-state calls skip retrace/recompile/reload."""
    key = id(nc)
    if key not in _CALL_CACHE:
        import jax
        from jax.sharding import Mesh, PartitionSpec
        from jax.experimental.shard_map import shard_map
        import concourse.mybir as mybir
        from concourse import bass2jax

        bass2jax.install_neuronx_cc_hook()
        partition_name = (nc.partition_id_tensor.name
                          if nc.partition_id_tensor else None)
        in_names, out_names, out_avals, zero_outs = [], [], [], []
        for alloc in nc.m.functions[0].allocations:
            if not isinstance(alloc, mybir.MemoryLocationSet):
                continue
            name = alloc.memorylocations[0].name
            if alloc.kind == "ExternalInput":
                if name != partition_name:
                    in_names.append(name)
            elif alloc.kind == "ExternalOutput":
                shape = tuple(alloc.tensor_shape)
                dtype = mybir.dt.np(alloc.dtype)
                out_names.append(name)
                out_avals.append(jax.core.ShapedArray(shape, dtype))
                zero_outs.append(np.zeros(shape, dtype))
        n_params = len(in_names)
        n_outs = len(out_avals)
        all_in = list(in_names) + list(out_names)
        if partition_name is not None:
            all_in.append(partition_name)
        donate = tuple(range(n_params, n_params + n_outs))

        def _body(*args):
            operands = list(args)
            if partition_name is not None:
                operands.append(bass2jax.partition_id_tensor())
            outs = bass2jax._bass_exec_p.bind(
                *operands,
                out_avals=tuple(out_avals),
                in_names=tuple(all_in),
                out_names=tuple(out_names),
                lowering_input_output_aliases=(),
                sim_require_finite=True,
                sim_require_nnan=True,
                nc=nc,
            )
            return tuple(outs)

        devices = jax.devices()[:8]
        mesh = Mesh(np.asarray(devices), ("core",))
        in_specs = (PartitionSpec("core"),) * (n_params + n_outs)
        out_specs = (PartitionSpec("core"),) * n_outs
        sharded = jax.jit(
            shard_map(_body, mesh=mesh, in_specs=in_specs,
                      out_specs=out_specs, check_rep=False),
            donate_argnums=donate, keep_unused=True)
        _CALL_CACHE[key] = (sharded, in_names, out_names, out_avals, zero_outs)

    sharded, in_names, out_names, out_avals, zero_outs = _CALL_CACHE[key]
    n_cores = 8
    concat_in = [
        np.concatenate([np.asarray(in_maps[c][nm]) for c in range(n_cores)], axis=0)
        for nm in in_names
    ]
    concat_zeros = [
        np.zeros((n_cores * z.shape[0], *z.shape[1:]), z.dtype) for z in zero_outs
    ]
    out_arrs = sharded(*concat_in, *concat_zeros)
    return [
        {nm: np.asarray(out_arrs[i]).reshape(n_cores, *out_avals[i].shape)[c]
         for i, nm in enumerate(out_names)}
        for c in range(n_cores)
    ]


def _run(x, lengths, wdict, T):
    nc = _get_program(T)
    in_maps = _prep_core_inputs(x, lengths, wdict, T)
    results = _spmd_call(nc, in_maps)
    B = x.shape[1]
    out = np.zeros((T, B, 2 * H), np.float32)
    for core in range(8):
        q, d = core // 2, core % 2
        hl = results[core]["OUT"].astype(np.float32).reshape(T, BL, H)
        if d:
            hl = hl[::-1]
        out[:, BL * q:BL * (q + 1), H * d:H * (d + 1)] = hl
    return out


def kernel(x, lengths, **weights):
    x = np.asarray(x, np.float32)
    lengths = np.asarray(lengths)
    wd = {k: np.asarray(v, np.float32) for k, v in weights.items()}
    return _run(x, lengths, wd, x.shape[0])
